# revision 1
# baseline (speedup 1.0000x reference)
"""Trainium2 Bass kernel for nn_APIHyperInputLayer (hypernetwork input layer).

Math (per branch, ally shown; enemy identical with F=28, E=11):
    h    = relu(feats @ w1 + b1)              [N, 64]
    w    = (h @ w2 + b2).reshape(N, F, 256)
    hid  = einsum('nf,nfo->no', feats, w)     [N, 256]
    out  = hid.reshape(B, E, 256).sum(1)      [B, 256]

Key restructurings:
  1. Avoid materializing w (335MB):
       hid.T = W2.T @ G,  G[(j,f), n] = relu(h)[n,j] * feats[n,f]
     with W2 = w2.reshape(64*F, 256) (j-major, f-fast).
  2. The entity sum commutes past W2:
       out[b, :] = sum_k W2[k, :] * Gs[k, b],  Gs[k, b] = sum_e G[k, (b,e)]
     so the big matmul contracts against the entity-POOLED Gs [HF, BC]
     instead of G [HF, N] — ~10x less PE work — and with Gs as the
     stationary operand the result lands directly as out[b, o] in PSUM,
     accumulating both branches into one [128, 256] tile (no final
     transposes / adds).

Per core (data-parallel over batch, BC=128 batches/core):
  - feats arrive host-transposed+replicated-ready as bf16 [F, N]; SBUF
    replicas at row-group bases 0/32/64/96 serve the 4-way tile_position
    packed expansion matmuls (for ally F=32 the same tile doubles as the
    elementwise multiplicand; enemy needs a separate F-strided quad).
  - w1 arrives host-packed by row-group base: base b holds the k-tiles
    t = b (mod 4), so one dense [128, 4*TH] DMA replaces 4 replicas.
  - expansion (bf16): pex(t) [TH, BW] = w1-slice.T @ featsT-slice per
    n-block (BW = 32 batches * E cols), 4 concurrent via row groups.
  - G(t) bf16 = max(pex, 0) * featsT-replica — routed across DVE
    (fused scalar_tensor_tensor), Pool (fused stt), and ACT relu + Pool
    mult, per ROUTE, to balance engine load.
  - Gs_f32[t][:, bb] = grouped entity reduce of G (DVE, 3D AP) then one
    ACT convert to bf16 per k-tile.
  - big matmul: out_psum[128b, 256o] += Gs_bf16[t].T @ W2[t], 32
    accumulating matmuls over both branches' k-tiles; copy + DMA out.

bf16 everywhere off the PE accumulators keeps total rel err ~1e-3
(tolerance 2e-2) while halving DMA and enabling 2x elementwise rates.

Biases are pinned to zero in this problem spec; the bias=True fallback
(2-way packing, K=F+1 ones-row for b1, entity-pooled feats @ B2 term
for b2) keeps the kernel mathematically complete for nonzero biases.
"""

import sys

if "/opt/trn_rl_repo" not in sys.path:
    sys.path.insert(0, "/opt/trn_rl_repo")

import numpy as np
import ml_dtypes

import concourse.mybir as mybir
from concourse import bacc
from concourse.tile import TileContext
from concourse.bass_utils import run_bass_kernel_spmd

F32 = mybir.dt.float32
BF16 = mybir.dt.bfloat16
AX = mybir.AxisListType
ALU = mybir.AluOpType
ACTF = mybir.ActivationFunctionType
BF = ml_dtypes.bfloat16

N_CORES = 8
B = 1024
OUT = 256
HID = 64

CFG = {
    "a": dict(F=32, E=10, TH=128),
    "e": dict(F=28, E=11, TH=112),
}
BC = B // N_CORES  # 128 batches per core
for _k, _c in CFG.items():
    _c["N"] = BC * _c["E"]           # rows per core (1280 / 1408)
    _c["HF"] = HID * _c["F"]         # contraction size (2048 / 1792)
    _c["KT"] = _c["HF"] // _c["TH"]  # 16 k-tiles, both branches
    _c["EH"] = _c["E"] // 2          # halved entity count (5 / 5)
    # entity-aligned n-blocks sized near the 512-f32 PSUM bank limit
    _bw = 32 * _c["E"]
    if _c["E"] == 10:
        _c["BLOCKS"] = [(0, 480), (480, 480), (960, 320)]
    else:
        _c["BLOCKS"] = [(0, 484), (484, 484), (968, 440)]
    assert sum(w for _, w in _c["BLOCKS"]) == _c["N"]

# Elementwise routing. Pool/GPSIMD cannot read PSUM (BIR verifier rule),
# so PSUM evacuation of the expansion output is split between DVE (fused
# relu*mult stt) and ACT (relu to bf16) + Pool (mult, SBUF-only): a
# block goes to DVE when (block_index % dve_den) < dve_num. Every k-tile
# is then entity-HALVED on Pool (strided pair-add, bf16). The first
# poff_a/poff_e k-tiles per branch are pooled by the PE (EH+1 strided-
# stationary accumulating matmuls into the output PSUM); the rest get a
# DVE grouped reduce (+ leftover add for odd E) and an ACT convert.
ROUTE = dict(dve_num=7, dve_den=12, poff_a=16, poff_e=16)


def _build_program(reps=1, bias=False, route=None):
    route = dict(ROUTE if route is None else route)
    pack = 2 if bias else 4
    gstep = 128 // pack
    bases = list(range(0, 128, gstep))
    kext = (lambda c: c["F"] + 1) if bias else (lambda c: c["F"])

    nc = bacc.Bacc("TRN2", debug=False)

    dr = {}
    for br in ("a", "e"):
        c = CFG[br]
        dr[f"x{br}"] = nc.dram_tensor(f"x{br}", [c["F"], c["N"]], BF16,
                                      kind="ExternalInput")
        dr[f"w1{br}"] = nc.dram_tensor(
            f"w1{br}", [bases[-1] + kext(c), (c["KT"] // pack) * c["TH"]],
            BF16, kind="ExternalInput")
        dr[f"w2{br}"] = nc.dram_tensor(f"w2{br}", [c["HF"], 256], BF16,
                                       kind="ExternalInput")
        if bias:
            dr[f"b2{br}"] = nc.dram_tensor(f"b2{br}", [c["F"], 256], BF16,
                                           kind="ExternalInput")
    if bias:
        dr["ones"] = nc.dram_tensor("ones", [1, 1408], BF16,
                                    kind="ExternalInput")
    out_dram = nc.dram_tensor("out", [BC, 256], F32, kind="ExternalOutput")

    with TileContext(nc) as tc:
        with (
            tc.tile_pool(name="const", bufs=1) as cpool,
            tc.tile_pool(name="tmp", bufs=4) as tpool,
            tc.tile_pool(name="g", bufs=12) as gpool,
            tc.tile_pool(name="psum", bufs=1, space="PSUM") as ppool,
        ):
          for _rep in range(reps):
            w1sb, w2sb, ext, quad, b2sb = {}, {}, {}, {}, {}
            for br in ("a", "e"):
                c = CFG[br]
                w1sb[br] = cpool.tile(
                    [bases[-1] + kext(c), (c["KT"] // pack) * c["TH"]],
                    BF16, name=f"w1{br}sb")
                w2sb[br] = [
                    cpool.tile([c["TH"], 256], BF16, name=f"w2{br}t{t}")
                    for t in range(c["KT"])
                ]
                ext[br] = cpool.tile([bases[-1] + kext(c), c["N"]], BF16,
                                     name=f"ext{br}")
                if bias or c["F"] != gstep:
                    quad[br] = cpool.tile([4 * c["F"], c["N"]], BF16,
                                          name=f"quad{br}")
                if bias:
                    b2sb[br] = cpool.tile([c["F"], 256], BF16,
                                          name=f"b2{br}sb")

            # ---- loads: w1 + featsT replicas first, split across both
            # HWDGE rings so the first expansion matmuls unblock early;
            # bulk w2 after, alternating rings ----
            # ACT is compute-loaded (relu path), so its HWDGE ring carries
            # only a small early slice (w1 + first replica per branch);
            # everything else issues from the SP ring.
            for br in ("a", "e"):
                c = CFG[br]
                nc.scalar.dma_start(w1sb[br], dr[f"w1{br}"][:, :])
                for i, base in enumerate(bases):
                    eng = nc.scalar if i == 0 else nc.sync
                    eng.dma_start(
                        ext[br][base: base + c["F"], :], dr[f"x{br}"][:, :])
                if bias:
                    for base in bases:
                        nc.scalar.dma_start(
                            ext[br][base + c["F"]: base + c["F"] + 1, :],
                            dr["ones"][:, : c["N"]])
                if br in quad:
                    for g in range(4):
                        nc.sync.dma_start(
                            quad[br][g * c["F"]:(g + 1) * c["F"], :],
                            dr[f"x{br}"][:, :])
                if bias:
                    nc.scalar.dma_start(b2sb[br], dr[f"b2{br}"][:, :])
            for br in ("a", "e"):
                c = CFG[br]
                for t in range(c["KT"]):
                    eng = nc.scalar if t % 4 == 3 else nc.sync
                    eng.dma_start(
                        w2sb[br][t],
                        dr[f"w2{br}"][t * c["TH"]:(t + 1) * c["TH"], :])

            poff = {"a": route["poff_a"], "e": route["poff_e"]}
            gfull, ghalf, gs32, gs16 = {}, {}, {}, {}
            for br in ("a", "e"):
                c = CFG[br]
                gfull[br] = [gpool.tile([c["TH"], c["N"]], BF16,
                                        name=f"g{br}{t}", tag="g")
                             for t in range(c["KT"])]
                ghalf[br] = [gpool.tile([c["TH"], BC * c["EH"]], BF16,
                                        name=f"gh{br}{t}", tag="gh")
                             for t in range(c["KT"])]
                gs32[br] = [cpool.tile([c["TH"], BC], F32,
                                       name=f"gs32{br}{t}")
                            if t >= poff[br] else None
                            for t in range(c["KT"])]
                gs16[br] = [cpool.tile([c["TH"], BC], BF16,
                                       name=f"gs16{br}{t}")
                            if t >= poff[br] else None
                            for t in range(c["KT"])]

            def mult_operand(br, lo, w):
                c = CFG[br]
                src = quad[br] if br in quad else ext[br]
                return src[: c["TH"], lo: lo + w]

            # ---- main pipeline: k-tile outer, n-block inner. All 4
            # blocks of a k-tile share one loaded weight slice, the wide
            # entity-reduce fires per k-tile (spreading DVE load evenly),
            # and output-PSUM matmuls are woven in LAG k-tiles behind so
            # the in-order PE queue never stalls the expansion feed. ----
            pout = ppool.tile([BC, 256], F32, name="pout", tag="pout")
            n_pout = sum(
                poff[br] * (CFG[br]["EH"] + CFG[br]["E"] % 2)
                + (CFG[br]["KT"] - poff[br])
                for br in ("a", "e"))
            pout_emitted = [0]
            pending = []

            def emit_pout(br, t):
                c = CFG[br]
                if t < poff[br]:
                    lhss = [ghalf[br][t].rearrange(
                        "p (b e) -> p b e", e=c["EH"])[:, :, e]
                        for e in range(c["EH"])]
                    if c["E"] % 2:  # odd E: unpaired entity from G
                        lhss.append(gfull[br][t].rearrange(
                            "p (b e) -> p b e", e=c["E"])[:, :, c["E"] - 1])
                else:
                    lhss = [gs16[br][t]]
                for lhs in lhss:
                    pout_emitted[0] += 1
                    nc.tensor.matmul(
                        pout, lhs, w2sb[br][t],
                        start=(pout_emitted[0] == 1),
                        stop=(pout_emitted[0] == n_pout and not bias),
                    )

            gi = [0]
            LAG = 2
            for br in ("a", "e"):
                c = CFG[br]
                kf = kext(c)
                for t in range(c["KT"]):
                    base = bases[t % pack]
                    for lo, w in c["BLOCKS"]:
                        pex = ppool.tile([c["TH"], w], F32,
                                         name="pex", tag="pex", bufs=6)
                        nc.tensor.matmul(
                            pex,
                            w1sb[br][base: base + kf,
                                     (t // pack) * c["TH"]:
                                     (t // pack + 1) * c["TH"]],
                            ext[br][base: base + kf, lo: lo + w],
                            start=True,
                            stop=True,
                            tile_position=(base, 0),
                        )
                        gt = gfull[br][t][:, lo: lo + w]
                        if gi[0] % route["dve_den"] < route["dve_num"]:
                            nc.vector.scalar_tensor_tensor(
                                gt, pex, 0.0, mult_operand(br, lo, w),
                                op0=ALU.max, op1=ALU.mult)
                        else:
                            tmp = tpool.tile([c["TH"], w], BF16,
                                             name="tmp", tag="tmp")
                            nc.scalar.activation(tmp, pex, ACTF.Relu)
                            nc.gpsimd.tensor_tensor(
                                gt, tmp, mult_operand(br, lo, w),
                                op=ALU.mult)
                        gi[0] += 1
                    # entity pair-halving on Pool (strided bf16 adds)
                    g3 = gfull[br][t].rearrange("p (b e) -> p b e",
                                                e=c["E"])
                    eh = c["EH"]
                    nc.gpsimd.tensor_tensor(
                        ghalf[br][t].rearrange("p (b e) -> p b e", e=eh),
                        g3[:, :, 0:eh], g3[:, :, eh:2 * eh], op=ALU.add)
                    if t >= poff[br]:
                        # grouped entity reduce on halves (DVE-only op)
                        nc.vector.tensor_reduce(
                            gs32[br][t],
                            ghalf[br][t].rearrange("p (b e) -> p b e",
                                                   e=eh),
                            axis=AX.X, op=ALU.add)
                        if c["E"] % 2:
                            nc.vector.tensor_tensor(
                                gs32[br][t], gs32[br][t],
                                g3[:, :, c["E"] - 1], op=ALU.add)
                        nc.scalar.copy(gs16[br][t], gs32[br][t])
                    pending.append((br, t))
                    if len(pending) > LAG:
                        emit_pout(*pending.pop(0))
            while pending:
                emit_pout(*pending.pop(0))
            if bias:
                # out += sum_e feats[., f] @ B2[f, :]  via entity-pooled feats
                for j, br in enumerate(("a", "e")):
                    c = CFG[br]
                    fsum = cpool.tile([c["F"], BC], F32, name=f"fsum{br}")
                    nc.vector.tensor_reduce(
                        fsum,
                        ext[br][: c["F"], :].rearrange(
                            "p (b e) -> p b e", e=c["E"]),
                        axis=AX.X, op=ALU.add)
                    fsum16 = cpool.tile([c["F"], BC], BF16,
                                        name=f"fsum16{br}")
                    nc.scalar.copy(fsum16, fsum)
                    nc.tensor.matmul(
                        pout, fsum16, b2sb[br],
                        start=False, stop=(j == 1))

            out_sb = cpool.tile([BC, 256], F32, name="out_sb")
            nc.scalar.copy(out_sb, pout)
            nc.sync.dma_start(out_dram[:, :], out_sb)

    nc.compile()
    return nc


def _pack_w1(w1, b1, F, TH, pack, bias):
    """Host-pack first-layer weights by row-group base: base b holds the
    k-tiles t congruent to b (mod pack), densely."""
    w1r = np.repeat(np.asarray(w1, dtype=np.float32), F, axis=1)  # [F, HF]
    kf = F + 1 if bias else F
    rows = (pack - 1) * (128 // pack) + kf
    kt = w1r.shape[1] // TH
    packed = np.zeros((rows, (kt // pack) * TH), dtype=np.float32)
    for t in range(kt):
        base = (t % pack) * (128 // pack)
        ti = t // pack
        packed[base: base + F, ti * TH:(ti + 1) * TH] = \
            w1r[:, t * TH:(t + 1) * TH]
        if bias:
            packed[base + F, ti * TH:(ti + 1) * TH] = np.repeat(
                np.asarray(b1, dtype=np.float32), F)[t * TH:(t + 1) * TH]
    return packed.astype(BF)


def _host_inputs(ally_features, enemy_features, wa1, ba1, wa2, ba2,
                 we1, be1, we2, be2, bias=False):
    pack = 2 if bias else 4
    shared = {
        "w1a": _pack_w1(wa1, ba1, 32, CFG["a"]["TH"], pack, bias),
        "w1e": _pack_w1(we1, be1, 28, CFG["e"]["TH"], pack, bias),
        "w2a": np.asarray(wa2, dtype=np.float32).reshape(2048, 256)
                 .astype(BF),
        "w2e": np.asarray(we2, dtype=np.float32).reshape(1792, 256)
                 .astype(BF),
    }
    if bias:
        shared["b2a"] = np.asarray(ba2, np.float32).reshape(32, 256)\
            .astype(BF)
        shared["b2e"] = np.asarray(be2, np.float32).reshape(28, 256)\
            .astype(BF)
        shared["ones"] = np.ones((1, 1408), dtype=BF)

    af = np.asarray(ally_features, dtype=np.float32)
    ef = np.asarray(enemy_features, dtype=np.float32)
    na, ne = CFG["a"]["N"], CFG["e"]["N"]
    in_maps = []
    for cix in range(N_CORES):
        m = dict(shared)
        m["xa"] = np.ascontiguousarray(
            af[cix * na:(cix + 1) * na].T).astype(BF)
        m["xe"] = np.ascontiguousarray(
            ef[cix * ne:(cix + 1) * ne].T).astype(BF)
        in_maps.append(m)
    return in_maps


_nc_cache = {}


def _get_nc(reps=1, **kw):
    key = (reps, tuple(sorted(kw.items())))
    if key not in _nc_cache:
        _nc_cache[key] = _build_program(reps, **kw)
    return _nc_cache[key]


def kernel(**inputs) -> np.ndarray:
    bias = any(
        np.any(np.asarray(inputs[k])) for k in ("ba1", "ba2", "be1", "be2")
    )
    nc = _get_nc(bias=bias)
    in_maps = _host_inputs(bias=bias, **inputs)
    res = run_bass_kernel_spmd(nc, in_maps, core_ids=list(range(N_CORES)))
    return np.concatenate([r["out"] for r in res.results], axis=0)


if __name__ == "__main__":
    import reference

    inputs = {k: np.asarray(v) for k, v in reference.setup_inputs().items()}
    expected = np.asarray(reference.reference(**inputs))
    actual = kernel(**inputs)
    denom = np.abs(expected).max()
    print("abs max err:", np.abs(actual - expected).max())
    print("rel err:", np.abs(actual - expected).max() / denom)



# revision 10
# speedup vs baseline: 3.4749x; 3.4749x over previous
"""Trainium2 Bass kernel for nn_APIHyperInputLayer (hypernetwork input layer).

Math (per branch, ally shown; enemy identical with F=28, E=11):
    h    = relu(feats @ w1 + b1)              [N, 64]
    w    = (h @ w2 + b2).reshape(N, F, 256)
    hid  = einsum('nf,nfo->no', feats, w)     [N, 256]
    out  = hid.reshape(B, E, 256).sum(1)      [B, 256]

Key identity: the entity-pooled contraction operand
    Gs[(j,f), b] = sum_e rh[(b,e), j] * feats[(b,e), f]
(with rh = relu(h)) is, for each 128-row chunk of entity-rows, a single
PE matmul against a batch-staircased copy of the features:
    out_f[j, b] = rh_chunk.T @ (feats_chunk * mask)   per feature f,
where mask[(b_loc,e), b'] = [b_loc == b'] scatters each batch's rows
into its own column. Rows are padded per chunk to whole batches
(CB = 128//E batches per chunk), so the staircase mask is one constant
[128, CB] tile and PSUM windows never straddle chunks.

This removes the expanded-G elementwise volume (the classic approach)
entirely: per chunk, one Pool broadcast-mult builds S = feats x mask
[128, F*CB], and two PE matmuls (even/odd feature parity, output
partitions 0-63 / 64-127 via tile_position) emit Gs slabs that are
copied straight into k-tile layout [(f%2)*64+j, f//2, b] (k = f*64+j,
matching a host-reordered W2R = w2.reshape(64,F,256).transpose(1,0,2)).
The big matmul is then 30 accumulating [128k,128b]^T @ [128k,256o]
matmuls into a single [128, 256] PSUM tile.

Per-iteration engine cost collapses from ~38us (all four engines
saturated: PE expansion 17.9us + PE pooling 18.8us, DVE/Pool/ACT full
of relu-mult/halving) to ~7-9us spread as: PE ~7.3us (h 1.5k + strips
8k + big 7.7k cols), Pool ~7us (S builds), DVE ~6us (relu + evac
share), ACT ~6us (evac share + out copy).

Biases (zero in this problem spec) are supported exactly: b1 rides an
extra ones-row in the h contraction; b2 contributes
sum_f B2[f,:] * fsum[f,b] with fsum from one tiny mask-matmul per
chunk plus a final [F,128]^T @ [F,256] accumulate.
"""

import sys

if "/opt/trn_rl_repo" not in sys.path:
    sys.path.insert(0, "/opt/trn_rl_repo")

import numpy as np
import ml_dtypes

import concourse.mybir as mybir
from concourse import bacc
from concourse import bass
from concourse.tile import TileContext
from concourse.bass_utils import run_bass_kernel_spmd

F32 = mybir.dt.float32
BF16 = mybir.dt.bfloat16
ALU = mybir.AluOpType
BF = ml_dtypes.bfloat16

N_CORES = 8
B = 1024
OUT = 256
HID = 64
BC = B // N_CORES  # 128 batches per core

CFG = {
    "a": dict(F=32, E=10),
    "e": dict(F=28, E=11),
}
for _c in CFG.values():
    _c["CB"] = 128 // _c["E"]                     # batches per chunk
    _c["RR"] = _c["CB"] * _c["E"]                 # real rows per chunk
    _c["NCH"] = -(-BC // _c["CB"])                # chunks per core
    _c["FH"] = _c["F"] // 2                       # features per parity
    _c["KT"] = _c["F"] * HID // 128               # k-tiles (16 / 14)
    _c["PERM"] = [f for par in (0, 1) for f in range(par, _c["F"], 2)]

# consts tensor column layout: w1a | w1e | mka | mke | mexpa | mexpe
_COL = {}
_off = 0
for _nm, _w in (("w1a", HID), ("w1e", HID),
                ("mka", CFG["a"]["CB"]), ("mke", CFG["e"]["CB"]),
                ("mexpa", CFG["a"]["CB"] * CFG["a"]["F"]),
                ("mexpe", CFG["e"]["CB"] * CFG["e"]["F"])):
    _COL[_nm] = (_off, _off + _w)
    _off += _w
CONST_W = _off

# engine routing: evacs alternate DVE/ACT (dve when i % den < num);
# S-builds go to DVE every s_dve-th chunk, else Pool.
ROUTE = dict(ev_num=2, ev_den=4, s_dve=3)


def _build_program(reps=1, bias=False, route=None):
    route = dict(ROUTE if route is None else route)
    nc = bacc.Bacc("TRN2", debug=False)

    dr = {}
    dr["cst"] = nc.dram_tensor("cst", [128, CONST_W], BF16,
                               kind="ExternalInput")
    for br in ("a", "e"):
        c = CFG[br]
        dr[f"xt{br}"] = nc.dram_tensor(
            f"xt{br}", [c["F"] + 1, c["NCH"] * 128], BF16, kind="ExternalInput")
        dr[f"xn{br}"] = nc.dram_tensor(
            f"xn{br}", [128, c["NCH"] * c["F"]], BF16, kind="ExternalInput")
        dr[f"w2{br}"] = nc.dram_tensor(
            f"w2{br}", [128, c["KT"] * OUT], BF16, kind="ExternalInput")
        if bias:
            dr[f"b2{br}"] = nc.dram_tensor(
                f"b2{br}", [c["F"], OUT], BF16, kind="ExternalInput")
    out_dram = nc.dram_tensor("out", [BC, OUT], F32, kind="ExternalOutput")

    n_big = sum(CFG[br]["KT"] for br in ("a", "e")) + (2 if bias else 0)

    with TileContext(nc) as tc:
        with (
            tc.tile_pool(name="const", bufs=1) as cpool,
            tc.tile_pool(name="rh", bufs=4) as rpool,
            tc.tile_pool(name="s", bufs=4) as spool,
            tc.tile_pool(name="psum", bufs=1, space="PSUM") as ppool,
        ):
          for _rep in range(reps):
            sb = {}
            sb["cst"] = cpool.tile([128, CONST_W], BF16, name="cst", bufs=2)
            for br in ("a", "e"):
                c = CFG[br]
                sb[f"xt{br}"] = cpool.tile(
                    [c["F"] + 1, c["NCH"] * 128], BF16, name=f"xt{br}",
                    bufs=2)
                sb[f"xn{br}"] = cpool.tile(
                    [128, c["NCH"] * c["F"]], BF16, name=f"xn{br}", bufs=2)
                sb[f"kt{br}"] = cpool.tile([128, c["KT"] * BC], BF16,
                                           name=f"kt{br}", bufs=2)
                sb[f"w2{br}"] = cpool.tile([128, c["KT"] * OUT], BF16,
                                           name=f"w2{br}", bufs=2)
                if bias:
                    sb[f"b2{br}"] = cpool.tile([c["F"], OUT], BF16,
                                               name=f"b2{br}", bufs=2)

            def cst(nm, rows=128):
                lo, hi = _COL[nm]
                return sb["cst"][:rows, lo:hi]

            # loads: small/early tensors first (consts/xt/xn feed the
            # first chunks), bulk w2 halves after, split across rings.
            nc.scalar.dma_start(sb["cst"], dr["cst"][:, :])
            for br in ("a", "e"):
                c = CFG[br]
                nc.sync.dma_start(sb[f"xt{br}"], dr[f"xt{br}"][:, :])
                eng = nc.scalar if br == "e" else nc.sync
                eng.dma_start(sb[f"xn{br}"], dr[f"xn{br}"][:, :])
                if bias:
                    nc.scalar.dma_start(sb[f"b2{br}"], dr[f"b2{br}"][:, :])
            for br in ("a", "e"):
                c = CFG[br]
                half = c["KT"] // 2 * OUT
                nc.sync.dma_start(sb[f"w2{br}"][:, :half],
                                  dr[f"w2{br}"][:, :half])
                nc.scalar.dma_start(sb[f"w2{br}"][:, half:],
                                    dr[f"w2{br}"][:, half:])

            pout = ppool.tile([BC, OUT], F32, name="pout", tag="pout")
            big_emitted = [0]
            ev_i = [0]
            s_i = [0]

            def emit_big(br, t):
                big_emitted[0] += 1
                nc.tensor.matmul(
                    pout, sb[f"kt{br}"][:, t * BC:(t + 1) * BC],
                    sb[f"w2{br}"][:, t * OUT:(t + 1) * OUT],
                    start=(big_emitted[0] == 1),
                    stop=(big_emitted[0] == n_big),
                )

            fsum_ps = {}
            if bias:
                for br in ("a", "e"):
                    fsum_ps[br] = ppool.tile([CFG[br]["F"], BC], F32,
                                             name=f"fsum{br}", tag="fsum")

            def emit_pair_h(br, pr):
                """h matmuls for chunk pair pr -> one [128, 128] psum."""
                c = CFG[br]
                n_in_pair = min(2, c["NCH"] - 2 * pr)
                hp = ppool.tile([128, n_in_pair * HID], F32, name="hps",
                                tag="h", bufs=3)
                for i in range(n_in_pair):
                    ch = 2 * pr + i
                    nc.tensor.matmul(
                        hp[:, i * HID:(i + 1) * HID],
                        sb[f"xt{br}"][:, ch * 128:(ch + 1) * 128],
                        cst(f"w1{br}", c["F"] + 1),
                        start=True, stop=True)
                return hp

            def emit_pair_relu(br, pr, hp):
                c = CFG[br]
                n_in_pair = min(2, c["NCH"] - 2 * pr)
                rh = rpool.tile([128, n_in_pair * HID], BF16, name="rh",
                                tag="rh")
                nc.vector.tensor_scalar(rh, hp, 0.0, None, op0=ALU.max)
                return rh

            def emit_pair_rest(br, pr, rh):
                c = CFG[br]
                CBc, FHc, Fc = c["CB"], c["FH"], c["F"]
                nch = min(2, c["NCH"] - 2 * pr)
                b0 = 2 * pr * CBc
                nbt = min(nch * CBc, BC - b0)   # total batches in pair

                strip = ppool.tile([128, nch * CBc * FHc], F32,
                                   name="strip", tag="strip", bufs=3)
                for i in range(nch):
                    ch = 2 * pr + i
                    # S[p, (b, f)] = xn[p, f] * mask[p, b], b-major so
                    # both operands are innermost-packed bf16 (DVE 2x).
                    s = spool.tile([128, CBc * Fc], BF16, name="s",
                                   tag=f"s{br}")
                    s3 = s.rearrange("p (b f) -> p b f", f=Fc)
                    xn1 = sb[f"xn{br}"][:, ch * Fc:(ch + 1) * Fc].rearrange(
                        "p (o f) -> p o f", o=1)
                    me3 = cst(f"mexp{br}").rearrange("p (b f) -> p b f",
                                                     f=Fc)
                    a0, a1 = bass.broadcast_tensor_aps(xn1, me3)
                    seng = nc.vector if (s_i[0] % route["s_dve"]
                                         == route["s_dve"] - 1) \
                        else nc.gpsimd
                    s_i[0] += 1
                    seng.tensor_tensor(s3, a0, a1, op=ALU.mult)
                    for par in (0, 1):
                        prt = slice(par * 64, par * 64 + 64)
                        nc.tensor.matmul(
                            strip[prt,
                                  i * CBc * FHc:(i + 1) * CBc * FHc],
                            rh[:, i * HID:(i + 1) * HID],
                            s3[:, :, par * FHc:(par + 1) * FHc],
                            start=True, stop=True,
                            tile_position=(0, par * 64))
                    if bias:
                        bc0 = ch * CBc
                        nb = min(CBc, BC - bc0)
                        nc.tensor.matmul(
                            fsum_ps[br][:, bc0:bc0 + nb],
                            sb[f"xn{br}"][:, ch * Fc:(ch + 1) * Fc],
                            cst(f"mk{br}")[:, :nb],
                            start=True, stop=True)
                # one evac for the whole pair and both parities: strip
                # cols are (chunk, b, fh), contiguous in b across the
                # pair; fh == k-tile index for either partition half.
                src = strip.rearrange("p (b f) -> p b f", f=FHc)[:, :nbt, :]
                kt3 = sb[f"kt{br}"].rearrange("p (t b) -> p t b", b=BC)
                dst = kt3[:, :, b0:b0 + nbt].rearrange("p t b -> p b t")
                if ev_i[0] % route["ev_den"] < route["ev_num"]:
                    nc.vector.tensor_scalar(dst, src, 0.0, None,
                                            op0=ALU.add)
                else:
                    nc.scalar.copy(dst, src)
                ev_i[0] += 1

            # software-pipelined emission: keep one pair of h-matmuls in
            # flight ahead so the in-order PE queue never waits on the
            # DVE relu; branch-a big matmuls interleave with branch-e
            # chunks.
            pending_bigs = []
            for br in ("a", "e"):
                c = CFG[br]
                npair = -(-c["NCH"] // 2)
                hps = {0: emit_pair_h(br, 0)}
                for pr in range(npair):
                    if pr + 1 < npair:
                        hps[pr + 1] = emit_pair_h(br, pr + 1)
                    rh = emit_pair_relu(br, pr, hps[pr])
                    emit_pair_rest(br, pr, rh)
                    for _ in range(3):
                        if pending_bigs:
                            emit_big(*pending_bigs.pop(0))
                pending_bigs = [(br, t) for t in range(c["KT"])]
            for br, t in pending_bigs:
                emit_big(br, t)
            if bias:
                for br in ("a", "e"):
                    c = CFG[br]
                    fs16 = cpool.tile([c["F"], BC], BF16, name=f"fs16{br}")
                    nc.scalar.copy(fs16, fsum_ps[br])
                    big_emitted[0] += 1
                    nc.tensor.matmul(
                        pout, fs16, sb[f"b2{br}"],
                        start=False, stop=(big_emitted[0] == n_big))

            out_sb = cpool.tile([BC, OUT], F32, name="out_sb")
            nc.scalar.copy(out_sb, pout)
            nc.sync.dma_start(out_dram[:, :], out_sb)

    nc.compile()
    return nc


def _host_inputs(ally_features, enemy_features, wa1, ba1, wa2, ba2,
                 we1, be1, we2, be2, bias=False):
    feats = {"a": np.asarray(ally_features, np.float32),
             "e": np.asarray(enemy_features, np.float32)}
    w1 = {"a": np.asarray(wa1, np.float32), "e": np.asarray(we1, np.float32)}
    b1 = {"a": np.asarray(ba1, np.float32), "e": np.asarray(be1, np.float32)}
    w2 = {"a": np.asarray(wa2, np.float32), "e": np.asarray(we2, np.float32)}
    b2 = {"a": np.asarray(ba2, np.float32), "e": np.asarray(be2, np.float32)}

    shared = {}
    cst = np.zeros((128, CONST_W), np.float32)
    for br in ("a", "e"):
        c = CFG[br]
        F, CB = c["F"], c["CB"]
        w1e = np.concatenate([w1[br], b1[br][None, :]], axis=0)
        cst[: F + 1, slice(*_COL[f"w1{br}"])] = w1e
        mask = np.zeros((128, CB), np.float32)
        for r in range(c["RR"]):
            mask[r, r // c["E"]] = 1.0
        cst[:, slice(*_COL[f"mk{br}"])] = mask
        cst[:, slice(*_COL[f"mexp{br}"])] = np.repeat(mask, F, axis=1)
        w2r = (w2[br].reshape(HID, F, OUT).transpose(1, 0, 2)
               .reshape(F * HID, OUT))
        shared[f"w2{br}"] = (
            w2r.reshape(c["KT"], 128, OUT).transpose(1, 0, 2)
            .reshape(128, c["KT"] * OUT).astype(BF))
        if bias:
            # rows follow the same parity-major feature order as xn/fsum
            shared[f"b2{br}"] = b2[br].reshape(F, OUT)[c["PERM"]].astype(BF)
    shared["cst"] = cst.astype(BF)

    in_maps = []
    for cix in range(N_CORES):
        m = dict(shared)
        for br in ("a", "e"):
            c = CFG[br]
            F, E, CB, NCH = c["F"], c["E"], c["CB"], c["NCH"]
            fc = feats[br][cix * BC * E:(cix + 1) * BC * E]  # [BC*E, F]
            xn = np.zeros((128, NCH * F), np.float32)
            xt = np.zeros((F + 1, NCH * 128), np.float32)
            for ch in range(NCH):
                b0 = ch * CB
                nb = min(CB, BC - b0)
                rows = fc[b0 * E:(b0 + nb) * E]            # [nb*E, F]
                xn[: nb * E, ch * F:(ch + 1) * F] = rows[:, c["PERM"]]
                xt[:F, ch * 128: ch * 128 + nb * E] = rows.T
                xt[F, ch * 128: ch * 128 + nb * E] = 1.0
            m[f"xn{br}"] = xn.astype(BF)
            m[f"xt{br}"] = xt.astype(BF)
        in_maps.append(m)
    return in_maps


_nc_cache = {}


def _get_nc(reps=1, **kw):
    key = (reps, tuple(sorted(kw.items())))
    if key not in _nc_cache:
        _nc_cache[key] = _build_program(reps, **kw)
    return _nc_cache[key]


def kernel(**inputs) -> np.ndarray:
    bias = any(
        np.any(np.asarray(inputs[k])) for k in ("ba1", "ba2", "be1", "be2")
    )
    nc = _get_nc(bias=bias)
    in_maps = _host_inputs(bias=bias, **inputs)
    res = run_bass_kernel_spmd(nc, in_maps, core_ids=list(range(N_CORES)))
    return np.concatenate([r["out"] for r in res.results], axis=0)


if __name__ == "__main__":
    import reference

    inputs = {k: np.asarray(v) for k, v in reference.setup_inputs().items()}
    expected = np.asarray(reference.reference(**inputs))
    actual = kernel(**inputs)
    denom = np.abs(expected).max()
    print("abs max err:", np.abs(actual - expected).max())
    print("rel err:", np.abs(actual - expected).max() / denom)
